# revision 32
# baseline (speedup 1.0000x reference)
"""Trainium2 Bass kernel for ASPP-attention (HASP convs + tailor linear attention).

Contract: kernel(**inputs) takes the FULL unsharded inputs, shards
batch-parallel across 8 NeuronCores (1 image per core), and returns the full
[8, 128, 128, 128] float32 output.

Algorithm per core (x: [128, 16384] f32):
  x_q   = concat_j relu(conv_j(x) * bn_alpha + bn_beta)      (4 dilated branches)
  Q     = wq x_q + bq ; K = wk x + bk ; V = wv x + bv
  Qn,Kn = l2-normalize over the 16 channels per pixel
  mat   = Kn V^T ; ksum = sum_n Kn ; vsum = sum_n V
  out   = gamma * (vsum + mat^T Qn) / (N + Qn.(ksum+eps))

Device mapping highlights:
  - convs in f32r (fp22): 28 matmuls per 512-px tile, 4-way column-tiled on
    the PE, PSUM accumulation; x held in a zero-padded (+18 cols wraparound)
    f32 layout so every tap reads a contiguous full-width window
  - K/V transposed per 128-px chunk via bf16 x-chunk-stationary matmuls so
    per-pixel L2 norms become free-dim reductions; Q the same but in f32r
  - vsum via linearity: vsum = wv @ (sum_n x) + N*bv  (one tiny fp32 matmul)
  - tailor denominator computed in transposed [128,128] layout, PE-transposed
    and DMA-reshuffled into a [1, N] row for the final broadcast multiply
"""

import sys

sys.path.insert(0, "/opt/trn_rl_repo")

import numpy as np
import ml_dtypes

import concourse.bass as bass
import concourse.bacc as bacc
import concourse.mybir as mybir
import concourse.tile as tile
from concourse.bass_utils import run_bass_kernel_spmd

B, C, H, W = 8, 128, 128, 128
N = H * W  # 16384
M = 16
NCORES = 8
BN_EPS = 1e-5
EPS = 1e-6

FP32 = mybir.dt.float32
FP32R = mybir.dt.float32r
BF16 = mybir.dt.bfloat16
ALU = mybir.AluOpType
ACT = mybir.ActivationFunctionType
AX = mybir.AxisListType

ROWS_PER_TILE = 4
TILE = ROWS_PER_TILE * W  # 512 px
NTILES = H // ROWS_PER_TILE  # 32
CHUNK = W  # 128 px = one image row
NCHUNK = N // CHUNK  # 128
PAD = 18  # conv taps reach +-18 columns
WP = W + PAD  # row pitch in the padded f32 x: data + trailing zero strip
XLEN = PAD + H * WP + PAD  # leading/trailing pads for shifted views

DILS = [None, 6, 12, 18]  # branch 0 is the 1x1 conv


def _branch_taps(j):
    """Taps for branch j, center first (center must open the PSUM group)."""
    if j == 0:
        return [4]
    return [4] + [t for t in range(9) if t != 4]


def build_nc():
    nc = bacc.Bacc(None)

    x_d = nc.declare_dram_parameter("x", [C, XLEN], BF16, isOutput=False)
    wconv_d = nc.declare_dram_parameter("wconv", [C, 9 * 4 * 32], BF16, isOutput=False)
    beta_d = nc.declare_dram_parameter("beta", [C, 1], FP32, isOutput=False)
    wqT_d = nc.declare_dram_parameter("wqT", [C, M], FP32R, isOutput=False)
    wkT_d = nc.declare_dram_parameter("wkT", [C, M], BF16, isOutput=False)
    wvT_d = nc.declare_dram_parameter("wvT", [C, C], BF16, isOutput=False)
    wvT32_d = nc.declare_dram_parameter("wvT32", [C, C], FP32, isOutput=False)
    bqr_d = nc.declare_dram_parameter("bqr", [C, M], FP32, isOutput=False)
    bkr_d = nc.declare_dram_parameter("bkr", [C, M], FP32, isOutput=False)
    bvr_d = nc.declare_dram_parameter("bvr", [C, C], FP32, isOutput=False)
    nbv_d = nc.declare_dram_parameter("nbv", [C, 1], FP32, isOutput=False)
    id32_d = nc.declare_dram_parameter("id32", [C, C], FP32, isOutput=False)
    gam_d = nc.declare_dram_parameter("gam", [C, 1], FP32, isOutput=False)
    onesr_d = nc.declare_dram_parameter("onesr", [1, C], FP32R, isOutput=False)
    xsum_d = nc.declare_dram_parameter("xsum", [C, 1], FP32, isOutput=False)
    out_d = nc.declare_dram_parameter("out", [C, N], FP32, isOutput=True)

    with tile.TileContext(nc) as tc:
        with (
            tc.tile_pool(name="const", bufs=1) as cpool,
            tc.tile_pool(name="big", bufs=1) as bpool,
            tc.tile_pool(name="matp", bufs=1, space="PSUM") as matpool,
        ):
            # ---- constants into SBUF ----
            wconv_s = cpool.tile_from(wconv_d[:, :])
            beta_s = cpool.tile_from(beta_d[:, :])
            wqT_s = cpool.tile_from(wqT_d[:, :])
            wkT_s = cpool.tile_from(wkT_d[:, :])
            wvT_s = cpool.tile_from(wvT_d[:, :])
            wvT32_s = cpool.tile_from(wvT32_d[:, :])
            bqr_s = cpool.tile_from(bqr_d[:, :])
            bkr_s = cpool.tile_from(bkr_d[:, :])
            bvr_s = cpool.tile_from(bvr_d[:, :])
            nbv_s = cpool.tile_from(nbv_d[:, :])
            id32_s = cpool.tile_from(id32_d[:, :])
            gam_s = cpool.tile_from(gam_d[:, :])
            ones_s = cpool.tile([C, 1], BF16)
            nc.vector.memset(ones_s[:, :], 1.0)
            epsb_s = cpool.tile([1, 1], FP32)
            nc.vector.memset(epsb_s[:, :], EPS)
            onesr_s = cpool.tile_from(onesr_d[:, :])

            # ---- persistent big buffers ----
            x_bf = bpool.tile([C, XLEN], BF16)  # padded bf16 x
            qnt_all = bpool.tile([C, NCHUNK, M], FP32)  # Qn^T per chunk
            srow_s = bpool.tile([1, N], FP32R)  # tailor row
            vsum_s = bpool.tile([C, 1], FP32)
            mat32_s = bpool.tile([M, C], FP32R)
            kse_s = bpool.tile([1, M], FP32R)  # ksum + EPS row

            mat_ps = matpool.tile([M, C + 1], FP32)  # mat | ksum accumulator

            # padded view: row y data at [PAD + y*WP, +W), zeros elsewhere
            xb3 = x_bf[:, PAD : PAD + H * WP].rearrange("p (h w) -> p h w", w=WP)

            # ---- load x (bf16, host-padded) + vsum from host xsum ----
            NBLK = 8
            RPB = H // NBLK  # 16 rows per block
            with (
                tc.tile_pool(name="ldps", bufs=1, space="PSUM") as ldps,
            ):
                for b in range(NBLK):
                    hi = PAD + (b + 1) * RPB * WP if b < NBLK - 1 else XLEN
                    nc.sync.dma_start(
                        out=x_bf[:, PAD + b * RPB * WP : hi] if b else x_bf[:, 0:hi],
                        in_=x_d[:, PAD + b * RPB * WP : hi] if b else x_d[:, 0:hi],
                    )
                xsum_s = cpool.tile_from(xsum_d[:, :])
                vs_ps = ldps.tile([C, 1], FP32)
                nc.tensor.matmul(
                    vs_ps[:, :], wvT32_s[:, :], xsum_s[:, :], start=True, stop=True
                )
                nc.scalar.activation(
                    vsum_s[:, :], vs_ps[:, :], ACT.Identity, bias=nbv_s[:, 0:1]
                )

            # ---- pass 1 ----
            wc4 = wconv_s[:, :].rearrange("p (t j o) -> p t j o", t=9, j=4)

            with (
                tc.tile_pool(name="cvps", bufs=2, space="PSUM") as cvps,
                tc.tile_pool(name="kqps", bufs=2, space="PSUM") as kqps,
                tc.tile_pool(name="vtps", bufs=2, space="PSUM") as vtps,
                tc.tile_pool(name="xq", bufs=2) as xqpool,
                tc.tile_pool(name="wk1", bufs=2) as wk1,
                tc.tile_pool(name="wk2", bufs=2) as wk2,
            ):
                prev = None  # skewed q-side state from tile t-1

                def q_side(st):
                    t0, xq_t, kq_t = st
                    for i in range(4):
                        nc.tensor.matmul(
                            kq_t[:, i, M : 2 * M],
                            xq_t[:, i * CHUNK : (i + 1) * CHUNK],
                            wqT_s[:, :],
                            start=(i == 0),
                            stop=(i == 3),
                        )
                    qt_sb = wk1.tile([C, 4, M], FP32, tag="qt")
                    nc.vector.scalar_tensor_tensor(
                        qt_sb[:, :, :],
                        kq_t[:, :, M : 2 * M],
                        0.0,
                        bqr_s[:, :].unsqueeze(1).to_broadcast([C, 4, M]),
                        op0=ALU.add,
                        op1=ALU.add,
                    )
                    q2m = wk2.tile([C, 4, M], FP32, tag="q2m")
                    nc.vector.tensor_tensor(
                        q2m[:, :, :], qt_sb[:, :, :], qt_sb[:, :, :], op=ALU.mult
                    )
                    q2 = wk2.tile([C, 4], FP32, tag="q2")
                    nc.vector.tensor_reduce(q2[:, :], q2m[:, :, :], axis=AX.X, op=ALU.add)
                    rq = wk2.tile([C, 4], FP32, tag="rq")
                    nc.scalar.sqrt(rq[:, :], q2[:, :])
                    nc.vector.reciprocal(rq[:, :], rq[:, :])
                    nc.vector.tensor_tensor(
                        qnt_all[:, 4 * t0 : 4 * t0 + 4, :],
                        qt_sb[:, :, :],
                        rq[:, :].unsqueeze(2).to_broadcast([C, 4, M]),
                        op=ALU.mult,
                    )

                for t in range(NTILES):
                    y0 = t * ROWS_PER_TILE
                    n0 = t * TILE

                    # --- convs: 4-way column-tiled f32r accumulation ---
                    cps = cvps.tile([C, ROWS_PER_TILE, W], FP32, tag="conv")
                    for j in range(4):
                        emit = []
                        for tp in _branch_taps(j):
                            ky, kx = tp // 3, tp % 3
                            d = DILS[j] or 0
                            dy, dx = d * (ky - 1), d * (kx - 1)
                            ra = max(0, -dy - y0)
                            rb = min(ROWS_PER_TILE, H - dy - y0)
                            if ra < rb:
                                emit.append((tp, dy, dx, ra, rb))
                        for k, (tp, dy, dx, ra, rb) in enumerate(emit):
                            xsh3 = x_bf[
                                :, PAD + dx : PAD + dx + H * WP
                            ].rearrange("p (h w) -> p h w", w=WP)
                            nc.tensor.matmul(
                                cps[32 * j : 32 * (j + 1), ra:rb, :].rearrange(
                                    "p a b -> p (a b)"
                                ),
                                wc4[:, tp, j, :],
                                xsh3[:, y0 + dy + ra : y0 + dy + rb, 0:W],
                                start=(k == 0),
                                stop=(k == len(emit) - 1),
                                tile_position=(0, 32 * j),
                            )
                    xq_t = xqpool.tile([C, TILE], FP32R, tag="xq")
                    nc.scalar.activation(
                        xq_t[:, :],
                        cps[:, :, :].rearrange("p a b -> p (a b)"),
                        ACT.Relu,
                        bias=beta_s[:, 0:1],
                    )

                    # --- k/v side for this tile (bf16) ---
                    kq_t = kqps.tile([C, 4, 2 * M], FP32, tag="kq")
                    vt_ps = vtps.tile([C, 4, C], FP32, tag="vt")
                    for i in range(4):
                        xch = xb3[:, 4 * t + i, 0:W]
                        nc.tensor.matmul(
                            kq_t[:, i, 0:M], xch, wkT_s[:, :],
                            start=(i == 0), stop=(i == 3),
                        )
                        nc.tensor.matmul(
                            vt_ps[:, i, :], xch, wvT_s[:, :],
                            start=(i == 0), stop=(i == 3),
                        )
                    kt_sb = wk1.tile([C, 4, M], FP32, tag="kt")
                    nc.vector.scalar_tensor_tensor(
                        kt_sb[:, :, :],
                        kq_t[:, :, 0:M],
                        0.0,
                        bkr_s[:, :].unsqueeze(1).to_broadcast([C, 4, M]),
                        op0=ALU.add,
                        op1=ALU.add,
                    )
                    vt_sb = wk1.tile([C, 4, C], BF16, tag="vt_sb")
                    nc.vector.scalar_tensor_tensor(
                        vt_sb[:, :, :],
                        vt_ps[:, :, :],
                        0.0,
                        bvr_s[:, :].unsqueeze(1).to_broadcast([C, 4, C]),
                        op0=ALU.add,
                        op1=ALU.add,
                    )
                    k2m = wk2.tile([C, 4, M], FP32, tag="k2m")
                    nc.vector.tensor_tensor(
                        k2m[:, :, :], kt_sb[:, :, :], kt_sb[:, :, :], op=ALU.mult
                    )
                    k2 = wk2.tile([C, 4], FP32, tag="k2")
                    nc.vector.tensor_reduce(k2[:, :], k2m[:, :, :], axis=AX.X, op=ALU.add)
                    rk = wk2.tile([C, 4], FP32, tag="rk")
                    nc.scalar.sqrt(rk[:, :], k2[:, :])
                    nc.vector.reciprocal(rk[:, :], rk[:, :])
                    knt_sb = wk1.tile([C, 4, M], BF16, tag="knt")
                    nc.vector.tensor_tensor(
                        knt_sb[:, :, :],
                        kt_sb[:, :, :],
                        rk[:, :].unsqueeze(2).to_broadcast([C, 4, M]),
                        op=ALU.mult,
                    )
                    for i in range(4):
                        g = 4 * t + i
                        nc.tensor.matmul(
                            mat_ps[:, 0:C],
                            knt_sb[:, i, :],
                            vt_sb[:, i, :],
                            start=(g == 0),
                            stop=False,
                        )
                        nc.tensor.matmul(
                            mat_ps[:, C : C + 1],
                            knt_sb[:, i, :],
                            ones_s[:, :],
                            start=False,
                            stop=(g == NCHUNK - 1),
                        )

                    if prev is not None:
                        q_side(prev)
                    prev = (t, xq_t, kq_t)
                q_side(prev)

            # ---- pass 1.5: mat/ksum extraction + tailor row ----
            with (
                tc.tile_pool(name="fps", bufs=1, space="PSUM") as fps,
                tc.tile_pool(name="fin", bufs=1) as fin,
            ):
                nc.scalar.copy(mat32_s[:, :], mat_ps[:, 0:C])
                ksum_sb = fin.tile([M, 1], FP32)
                nc.vector.tensor_copy(ksum_sb[:, :], mat_ps[:, C : C + 1])
                ksT_ps = fps.tile([1, M], FP32)
                nc.tensor.transpose(ksT_ps[:, :], ksum_sb[:, :], id32_s[0:M, 0:M])
                nc.scalar.activation(kse_s[:, :], ksT_ps[:, :], ACT.Identity, bias=epsb_s[0:1, 0:1])

                # physically replicate kse across partitions via a K=1 matmul
                kse_rep_ps = fps.tile([C, M], FP32)
                nc.tensor.matmul(
                    kse_rep_ps[:, :],
                    onesr_s[:, :],
                    kse_s[:, :],
                    start=True,
                    stop=True,
                )
                kse_rep = fin.tile([C, M], FP32)
                nc.vector.tensor_copy(kse_rep[:, :], kse_rep_ps[:, :])

                prod = fin.tile([C, NCHUNK, M], FP32)
                nc.vector.tensor_tensor(
                    prod[:, :, :],
                    qnt_all[:, :, :],
                    kse_rep[:, :].unsqueeze(1).to_broadcast([C, NCHUNK, M]),
                    op=ALU.mult,
                )
                tT = fin.tile([C, NCHUNK], FP32)
                nc.vector.tensor_reduce(tT[:, :], prod[:, :, :], axis=AX.X, op=ALU.add)
                nc.vector.tensor_scalar_add(tT[:, :], tT[:, :], float(N))
                sT = fin.tile([C, NCHUNK], FP32)
                nc.vector.reciprocal(sT[:, :], tT[:, :])
                sT_ps = fps.tile([NCHUNK, C], FP32)
                nc.tensor.transpose(sT_ps[:, :], sT[:, :], id32_s[:, :])
                s_cm = fin.tile([NCHUNK, C], FP32R)
                nc.scalar.copy(s_cm[:, :], sT_ps[:, :])
                nc.sync.dma_start(
                    out=srow_s[:, :].rearrange("o (a b) -> o a b", b=CHUNK),
                    in_=s_cm[:, :],
                )

            # ---- pass 2 ----
            with (
                tc.tile_pool(name="p2ps", bufs=2, space="PSUM") as p2ps,
                tc.tile_pool(name="p2sb", bufs=3) as p2sb,
            ):
                for t in range(NTILES):
                    n0 = t * TILE
                    tp_ps = p2ps.tile([M, TILE], FP32, tag="tp")
                    for i in range(4):
                        nc.tensor.transpose(
                            tp_ps[:, i * CHUNK : (i + 1) * CHUNK],
                            qnt_all[:, 4 * t + i, :],
                            id32_s[:, :],
                        )
                    qn_sb = p2sb.tile([M, TILE], FP32R, tag="qn")
                    nc.scalar.copy(qn_sb[:, :], tp_ps[:, :])
                    p_ps = p2ps.tile([C, TILE], FP32, tag="p")
                    nc.tensor.matmul(
                        p_ps[:, :],
                        mat32_s[:, :],
                        qn_sb[:, :],
                        start=True,
                        stop=True,
                    )
                    ms_sb = p2sb.tile([C, TILE], FP32, tag="ms")
                    nc.scalar.activation(
                        ms_sb[:, :], p_ps[:, :], ACT.Identity, bias=vsum_s[:, 0:1]
                    )
                    sbc_ps = p2ps.tile([C, TILE], FP32, tag="sbc")
                    nc.tensor.matmul(
                        sbc_ps[:, :],
                        onesr_s[:, :],
                        srow_s[0:1, n0 : n0 + TILE],
                        start=True,
                        stop=True,
                    )
                    o_sb = p2sb.tile([C, TILE], FP32, tag="o")
                    nc.vector.scalar_tensor_tensor(
                        o_sb[:, :],
                        ms_sb[:, :],
                        gam_s[:, 0:1],
                        sbc_ps[:, :],
                        op0=ALU.mult,
                        op1=ALU.mult,
                    )
                    nc.sync.dma_start(out=out_d[:, n0 : n0 + TILE], in_=o_sb[:, :])

    if not nc.is_finalized():
        nc.finalize()
    return nc


def _prep_shared(w1, w2, w3, w4, bn_scale, bn_bias, bn_mean, bn_var,
                 wq, bq, wk, bk, wv, bv, gamma):
    bf = ml_dtypes.bfloat16
    alpha = bn_scale / np.sqrt(bn_var + BN_EPS)  # [4, 32]
    beta = bn_bias - bn_mean * alpha  # [4, 32]
    ws = [w1, w2, w3, w4]
    wconv = np.zeros((C, 9, 4, 32), np.float32)
    for j in range(4):
        w = ws[j]  # [32, 128, kh, kw]
        if j == 0:
            wconv[:, 4, 0, :] = (w[:, :, 0, 0] * alpha[0][:, None]).T
        else:
            for tp in range(9):
                ky, kx = tp // 3, tp % 3
                wconv[:, tp, j, :] = (w[:, :, ky, kx] * alpha[j][:, None]).T
    shared = {
        "wconv": wconv.reshape(C, 9 * 4 * 32).astype(bf),
        "beta": beta.reshape(C, 1).astype(np.float32),
        "wqT": np.ascontiguousarray(wq[:, :, 0, 0].T, np.float32),
        "wkT": wk[:, :, 0, 0].T.astype(bf),
        "wvT": wv[:, :, 0, 0].T.astype(bf),
        "wvT32": np.ascontiguousarray(wv[:, :, 0, 0].T, np.float32),
        "bqr": np.tile(bq[None, :], (C, 1)).astype(np.float32),
        "bkr": np.tile(bk[None, :], (C, 1)).astype(np.float32),
        "bvr": np.tile(bv[None, :], (C, 1)).astype(np.float32),
        "nbv": (float(N) * bv).reshape(C, 1).astype(np.float32),
        "id32": np.eye(C, dtype=np.float32),
        "gam": np.full((C, 1), float(np.asarray(gamma).reshape(-1)[0]), np.float32),
        "onesr": np.ones((1, C), np.float32),
    }
    return shared


_NC_CACHE = {}


def get_nc():
    if "nc" not in _NC_CACHE:
        _NC_CACHE["nc"] = build_nc()
    return _NC_CACHE["nc"]


def make_in_maps(x, **kw):
    shared = _prep_shared(**kw)
    in_maps = []
    bf = ml_dtypes.bfloat16
    for b in range(NCORES):
        m = dict(shared)
        xb = np.asarray(x[b], np.float32)
        xp = np.zeros((C, XLEN), bf)
        xp[:, PAD : PAD + H * WP].reshape(C, H, WP)[:, :, 0:W] = xb.astype(bf)
        m["x"] = xp
        m["xsum"] = xb.reshape(C, N).sum(1, dtype=np.float64).astype(
            np.float32
        ).reshape(C, 1)
        in_maps.append(m)
    return in_maps


def kernel(x, w1, w2, w3, w4, bn_scale, bn_bias, bn_mean, bn_var,
           wq, bq, wk, bk, wv, bv, gamma):
    nc = get_nc()
    in_maps = make_in_maps(
        x, w1=w1, w2=w2, w3=w3, w4=w4, bn_scale=bn_scale, bn_bias=bn_bias,
        bn_mean=bn_mean, bn_var=bn_var, wq=wq, bq=bq, wk=wk, bk=bk,
        wv=wv, bv=bv, gamma=gamma,
    )
    res = run_bass_kernel_spmd(nc, in_maps, list(range(NCORES)))
    out = np.stack([res.results[b]["out"].reshape(C, H, W) for b in range(NCORES)])
    return out.astype(np.float32)
